# revision 73
# baseline (speedup 1.0000x reference)
"""Multi-head attention (dense transformer block) on 8 TRN2 NeuronCores.

Sharding: 8 cores = 4 batches x 2 head-halves.
  core c: batch b = c // 2, head half H = c % 2 (heads H*8 .. H*8+8).
  Each core computes attention for its 8 heads of its batch plus the
  partial final projection (row-shard of Wo); the host sums core pairs
  and adds the output bias in the same epilogue.

Per-core kernel. All SBUF-resident operands are bf16 (halves the input
DMA and enables FWL weight loads); PSUM accumulation stays fp32.

  0. Load pre-transposed x^T (host supplies bf16 x^T) into SBUF [e, s].
  1. Per 2-head group g: QT_g/KT_g [128, 2048] in [d, s] layout
     (wq/wk projections emitted sj-interleaved so attention's first ki
     chunks unblock early); V for 4 heads at a time in [s, d+1] layout
     with a ones column per head block.
  2. Attention per head PAIR (2g, 2g+1) and q-chunk of 512:
     scoresT [k, q]: two K=64 matmuls (base partitions 0/64) into one
     pair psum tile [128, (headA 512 | headB 512)], one ACT exp ->
     bf16 et (scale=1/8; no max subtraction: |score/8| <~ 6).
     attn@V runs FLIPPED: out[q, d] with et 128-q slices as lhsT and
     V' [k, 65] as rhs -> 8 matmuls of ap_size 65 per ki instead of 2
     of 512 (PE cost is output-free-size per accumulation step, so
     this halves attn@V PE time).  The ones column of V' lands the
     softmax denominator at column 64 of the SAME partition as its q
     row.  Only the first matmul per aps tile carries start=True (a
     start zeroes the whole 2KB psum zero-region); the other q-tiles'
     ki==0 matmuls overwrite via the pending-zero bytes.
     attn@V trails scores/exp by LAG=2 ki steps (a deferred job
     queue), so the in-order PE stream never parks on an exp wait
     while later scores could keep ACT (the 1038ns/ki bottleneck
     engine) saturated -- this also pipelines chunk and group
     boundaries through the normalize's psum-ring WAR.
  3. Normalize off the critical path: one DVE reciprocal per head
     (4 denominators via a strided psum view), then per q-tile a
     per-partition tensor_scalar multiply into a [128, (dA|dB)] bf16
     staging tile; ONE identity matmul transposes both heads' tiles
     back to [d, q] (psum), evacuated into outT[g] [hd, s] by DVE.
     The very last chunk routes alternate ops through the then-idle
     ACT engine.
  4. final: out[s,e] = sum_hd outT[hd,s]^T @ Wo[hd,e] -> DRAM in bf16
     (bias add + f32 upcast in the host pair-sum epilogue).  Groups
     0-2's contribution to the last four s-tiles is pre-accumulated
     into SBUF partials during group 3's exp-wait bubbles and
     re-injected via an identity matmul, so the post-attention tail
     only runs group 3's matmul per tile.

  Scheduling: the tile scheduler hoists ready work into PE bubbles,
  so each group builds only KT sj0/sj1 + QT sj0 up front and defers
  KT sj2/3 + QT sj1-3 (and V' half-builds) onto explicit ki-marks
  inside its own chunks; group 0 software-pipelines its first chunk
  against the 4 big per-sj xT DMAs (issued critical-first).
"""

import numpy as np

EMBED = 1024
HEADS = 16
HEAD_DIM = 64
SEQ = 2048
BATCH = 4
N_CORES = 8

LOCAL_HEADS = 8
N_GROUPS = 4
WCOLS = LOCAL_HEADS * HEAD_DIM  # 512

P = 128
NS = SEQ // P    # 16
NE = EMBED // P  # 8
VB = HEAD_DIM + 1  # 65
QC = 512         # q-chunk
NQ = SEQ // QC   # 4
NT4 = QC // P    # 4 q-tiles per q-chunk

TIMING_REPEATS = 16

_cache = {}


def _emit(nc, tc, tile, mybir, d):
    import os

    from concourse import masks

    f32 = mybir.dt.float32
    bf16 = mybir.dt.bfloat16
    EXP = mybir.ActivationFunctionType.Exp
    KDBG = bool(os.environ.get("KDBG")) and "dbg_aps" in d

    with (
        tc.tile_pool(name="const", bufs=1) as const_pool,
        tc.tile_pool(name="xt", bufs=1) as xt_pool,
        tc.tile_pool(name="v", bufs=1) as v_pool,
        tc.tile_pool(name="qk", bufs=2) as qk_pool,
        tc.tile_pool(name="wst", bufs=1) as wst_pool,
        tc.tile_pool(name="ps_s", bufs=2, space="PSUM") as ps_s,
        tc.tile_pool(name="ps_p", bufs=2, space="PSUM") as ps_p,
        tc.tile_pool(name="ps_a", bufs=2, space="PSUM") as ps_a,
    ):
        def load_wv(half):
            wvt = wst_pool.tile([P, NE * 256], bf16, tag="wv", name="wvt")
            wv_v = d["wv"][:].rearrange("(e p) c -> p e c", e=NE, p=P)
            nc.sync.dma_start(
                out=wvt[:].rearrange("p (e c) -> p e c", e=NE, c=256),
                in_=wv_v[:, :, half * 256:(half + 1) * 256],
            )
            return wvt

        def load_wqk2(name, pair):
            """One DMA loads wq/wk columns for TWO groups (256 cols): the
            512B contiguous runs dodge the <512B DMA read-modify-write
            penalty and halve the serialized HWDGE issue count."""
            wt = wst_pool.tile(
                [P, NE * 256], bf16, tag="wqk", bufs=2, name="wqk"
            )
            w_v = d[name][:].rearrange("(e p) c -> p e c", e=NE, p=P)
            nc.sync.dma_start(
                out=wt[:].rearrange("p (e c) -> p e c", e=NE, c=256),
                in_=w_v[:, :, pair * 256:(pair + 1) * 256],
            )
            return wt

        def wqk_views(wt, g):
            sub = g % 2
            return [
                wt[:, ei * 256 + sub * P: ei * 256 + (sub + 1) * P]
                for ei in range(NE)
            ]

        xt_big = xt_pool.tile([P, NE * SEQ], bf16, tag="xt", name="xt_big")
        xt_view = xt_big[:].rearrange("p (e s) -> p e s", e=NE, s=SEQ)
        x_dram = d["xt"][:].rearrange("(e p) s -> p e s", e=NE, p=P)

        def load_xt(sj):
            # one 1MB DMA per s-chunk: all 8 e-blocks at once (1KB runs)
            nc.sync.dma_start(
                out=xt_view[:, :, sj * 512:(sj + 1) * 512],
                in_=x_dram[:, :, sj * 512:(sj + 1) * 512],
            )

        # Critical-path-first DMA order (HWDGE issue AND the transfer
        # engine pool are serialized): wk then xT chunk 0 gate the first
        # KT build -> first scores; wq next (QT sj0), then V weights,
        # then the remaining xT chunks.
        load_xt(0)
        wqk_pre = {"wk": load_wqk2("wk", 0)}
        wqk_pre["wq"] = load_wqk2("wq", 0)
        wvt_pre = load_wv(0)
        for sj in range(1, 4):
            load_xt(sj)

        def xt_blk(ei, s0, slen):
            return xt_big[:, ei * SEQ + s0: ei * SEQ + s0 + slen]

        # V': [p][si][h][b=65] bf16, ones at col 64
        vp = v_pool.tile([P, NS * LOCAL_HEADS * VB], bf16, tag="vp", name="vp")
        vp_r = vp[:].rearrange(
            "p (s h b) -> p s h b", s=NS, h=LOCAL_HEADS, b=VB
        )
        ones_blocks = NS * LOCAL_HEADS
        ones_view = vp[:].rearrange(
            "p (blk c) -> p blk c", blk=ones_blocks, c=VB
        )[:, :, HEAD_DIM:HEAD_DIM + 1]

        ones128 = const_pool.tile([P, P], f32, tag="ones", name="ones128")
        nc.gpsimd.memset(ones128[:], 1.0)
        ident = const_pool.tile([P, P], bf16, tag="ident", name="ident")
        masks.make_identity(nc, ident[:])
        # warm the ACT exp table set (~2.7us ACT_TABLE_LOAD) during the
        # DMA-bound startup instead of at the first real softmax exp.
        warm = const_pool.tile([1, 1], f32, tag="warm", name="warm")
        nc.scalar.activation(warm[:], ones128[0:1, 0:1], EXP)
        nc.vector.tensor_copy(
            ones_view,
            ones128[:].rearrange("p (a b) -> p a b", a=P, b=1)[
                :, 0:ones_blocks, :
            ],
        )

        # PE p-state warmup: burn the 3us cold-clock ramp on dummy identity
        # matmuls while the first DMAs are in flight.
        wps = ps_s.tile([P, 1024], f32, tag="s", name="wps")
        for _ in range(52):
            nc.tensor.matmul(
                wps[:, 0:P], ident[:], ident[:],
                start=True, stop=True, skip_group_check=True,
            )

        with (
            tc.tile_pool(name="exp", bufs=1) as exp_pool,
            tc.tile_pool(name="small", bufs=1) as small_pool,
            tc.tile_pool(name="outt", bufs=1) as outt_pool,
        ):
            outt_tiles = [
                outt_pool.tile([P, SEQ], bf16, tag=f"outt{g}", name=f"outt{g}")
                for g in range(N_GROUPS)
            ]

            # dedicated wo tiles, loaded at group 2 so the partial final
            # projections can start as soon as groups 0-2's outT is done
            wo_tiles = []

            def load_wo():
                for j in range(2):
                    wo = wst_pool.tile(
                        [P, SEQ], bf16, tag="wo", name="wo", bufs=2
                    )
                    wo_tiles.append(wo)
                    for jj in range(2):
                        c = 2 * j + jj
                        nc.sync.dma_start(
                            out=wo[:, jj * 1024:(jj + 1) * 1024],
                            in_=d["wo"][c * P:(c + 1) * P, :],
                        )

            # Tail shortener: groups 0-2's contribution to the LAST four
            # s-tiles is pre-accumulated into SBUF during group 3's
            # exp-wait bubbles; the post-attention tail then only runs the
            # group-3 matmul + a DVE add per (si, ej).
            partials = {}

            def partial_final(si, ej):
                pt = ps_p.tile([P, 512], f32, tag="p", name="pt")
                for c in range(3):
                    nc.tensor.matmul(
                        pt[:, 0:512],
                        outt_tiles[c][:, si * P:(si + 1) * P],
                        wo_tiles[c // 2][:, (c % 2) * 1024 + ej * 512:
                                         (c % 2) * 1024 + (ej + 1) * 512],
                        start=(c == 0),
                        stop=(c == 2),
                        skip_group_check=True,
                    )
                part = small_pool.tile(
                    [P, 512], bf16, tag="part", name="part", bufs=16
                )
                nc.vector.tensor_copy(part[:], pt[:, 0:512])
                partials[(si, ej)] = part

            def final_proj(si_range):
                # ps_p only: the ps_s ring is serialized behind the whole
                # attention stream, which would block the si<8 half from
                # weaving into group 3's ACT-bound bubbles.  Stores go out
                # per ej half so the last DMA chain starts earlier.
                for si in si_range:
                    ot = exp_pool.tile(
                        [P, 1024], bf16, tag="ot", name="ot", bufs=8
                    )
                    for ej in range(2):
                        part = partials.get((si, ej))
                        if part is not None and ej:
                            # scores psum is free at the tail: widen the
                            # effective pt ring so evacs don't serialize
                            # behind the 2-slot ps_p ring
                            pt = ps_s.tile(
                                [P, 1024], f32, tag="s", name="pts"
                            )[:, 0:512]
                        else:
                            pt = ps_p.tile([P, 512], f32, tag="p", name="pt")
                        if part is not None:
                            # inject the groups-0..2 partial into the
                            # accumulation group: identity @ part == part
                            nc.tensor.matmul(
                                pt[:, 0:512],
                                ident[:],
                                part[:],
                                start=True,
                                stop=False,
                                skip_group_check=True,
                            )
                        crange = range(3, 4) if part is not None else range(4)
                        # bias is folded into the host pair-sum epilogue
                        for c in crange:
                            nc.tensor.matmul(
                                pt[:, 0:512],
                                outt_tiles[c][:, si * P:(si + 1) * P],
                                wo_tiles[c // 2][:, (c % 2) * 1024 + ej * 512:
                                                 (c % 2) * 1024 + (ej + 1) * 512],
                                start=(part is None and c == 0),
                                stop=(c == 3),
                                skip_group_check=True,
                            )
                        if part is not None:
                            # ACT is idle after the last exp: evacuate the
                            # tail halves there so DVE isn't the tail chain
                            eng = nc.vector.tensor_copy if ej == 0 else (
                                lambda o, i: nc.scalar.activation(
                                    o, i, mybir.ActivationFunctionType.Copy
                                )
                            )
                            eng(ot[:, ej * 512:(ej + 1) * 512], pt[:, 0:512])
                        else:
                            nc.vector.tensor_copy(
                                ot[:, ej * 512:(ej + 1) * 512], pt[:, 0:512]
                            )
                        nc.sync.dma_start(
                            out=d["out"][si * P:(si + 1) * P,
                                         ej * 512:(ej + 1) * 512],
                            in_=ot[:, ej * 512:(ej + 1) * 512],
                        )

            def normalize_pair(apsA, apsB, g, q0, nqt=NT4, tail=False):
                """Normalize + transpose both heads of the pair for one
                q-chunk.  aps tiles are [128, 512] psum: per q-tile qt a
                [128, 65] region at column qt*128 (col 64 = denominator,
                same partition as its q row)."""
                dbg_here = KDBG and g == 0 and q0 == QC
                recs = []
                for aps in (apsA, apsB):
                    rec = small_pool.tile(
                        [P, NT4], f32, tag="rec", name="rec", bufs=6
                    )
                    dview = aps[:].rearrange(
                        "p (q c) -> p q c", q=NT4, c=P
                    )[:, 0:nqt, HEAD_DIM:HEAD_DIM + 1].rearrange(
                        "p q c -> p (q c)"
                    )
                    nc.vector.reciprocal(rec[:, 0:nqt], dview)
                    recs.append(rec)
                if dbg_here:
                    nc.sync.dma_start(out=d["dbg_rec"][:], in_=recs[0][:])
                stgs = []
                for qt in range(nqt):
                    stg = small_pool.tile(
                        [P, P], bf16, tag="stg", name="stg", bufs=10
                    )
                    # tail (very last chunk): odd q-tiles ride the now-
                    # idle ACT engine (Copy with per-partition scale) so
                    # the post-attention normalize chain isn't DVE-serial
                    for sub, (aps, rec) in enumerate(zip((apsA, apsB), recs)):
                        if tail and qt % 2:
                            nc.scalar.activation(
                                stg[:, sub * HEAD_DIM:(sub + 1) * HEAD_DIM],
                                aps[:, qt * P:qt * P + HEAD_DIM],
                                mybir.ActivationFunctionType.Copy,
                                scale=rec[:, qt:qt + 1],
                            )
                        else:
                            nc.vector.tensor_scalar_mul(
                                stg[:, sub * HEAD_DIM:(sub + 1) * HEAD_DIM],
                                aps[:, qt * P:qt * P + HEAD_DIM],
                                rec[:, qt:qt + 1],
                            )
                    stgs.append(stg)
                if dbg_here:
                    for qt in range(NT4):
                        nc.sync.dma_start(
                            out=d["dbg_stg"][:, qt * P:(qt + 1) * P],
                            in_=stgs[qt][:],
                        )
                for qt in range(nqt):
                    # one identity matmul transposes both heads' 64-col
                    # halves: out rows 0..63 = headA^T, 64..127 = headB^T
                    tp = ps_a.tile([P, 512], f32, tag="a", name="tp")
                    nc.tensor.matmul(
                        tp[:, 0:P],
                        stgs[qt][:],
                        ident[:],
                        start=True,
                        stop=True,
                        skip_group_check=True,
                    )
                    if tail and qt % 2:
                        nc.scalar.activation(
                            outt_tiles[g][:, q0 + qt * P: q0 + (qt + 1) * P],
                            tp[:, 0:P],
                            mybir.ActivationFunctionType.Copy,
                        )
                    else:
                        nc.vector.tensor_copy(
                            outt_tiles[g][:, q0 + qt * P: q0 + (qt + 1) * P],
                            tp[:, 0:P],
                        )

            def build_v_chunk(si_range, wvt, h0):
                for si in si_range:
                    pt = ps_p.tile([P, 512], f32, tag="p", name="pt")
                    for ei in range(NE):
                        nc.tensor.matmul(
                            pt[:, 0:256],
                            xt_blk(ei, si * P, P),
                            wvt[:, ei * 256:(ei + 1) * 256],
                            start=(ei == 0),
                            stop=(ei == NE - 1),
                        )
                    dst = vp_r[:, si, h0:h0 + 4, 0:HEAD_DIM]
                    nc.vector.tensor_copy(
                        dst,
                        pt[:, 0:256].rearrange(
                            "p (h b) -> p h b", h=4, b=HEAD_DIM
                        ),
                    )

            def build_v2_chunk(si_range, wvt, h0, hh):
                # two-head V' build: halves the PE cost on group 0's
                # PE-bound startup path (heads 2-3 defer to qc1/qc2)
                for si in si_range:
                    pt = ps_p.tile([P, 512], f32, tag="p", name="pt")
                    for ei in range(NE):
                        nc.tensor.matmul(
                            pt[:, 0:P],
                            xt_blk(ei, si * P, P),
                            wvt[:, ei * 256 + hh * P:
                                ei * 256 + (hh + 1) * P],
                            start=(ei == 0),
                            stop=(ei == NE - 1),
                        )
                    dst = vp_r[:, si, h0 + 2 * hh:h0 + 2 * hh + 2, 0:HEAD_DIM]
                    nc.vector.tensor_copy(
                        dst,
                        pt[:, 0:P].rearrange(
                            "p (h b) -> p h b", h=2, b=HEAD_DIM
                        ),
                    )

            def build_qk_chunk(sj, qkt, wts, names=("wk", "wq")):
                for name in names:
                    pt = ps_p.tile([P, 512], f32, tag="p", name="pt")
                    for ei in range(NE):
                        nc.tensor.matmul(
                            pt[:, 0:512],
                            wts[name][ei],
                            xt_blk(ei, sj * 512, 512),
                            start=(ei == 0),
                            stop=(ei == NE - 1),
                        )
                    nc.vector.tensor_copy(
                        qkt[name][:, sj * 512:(sj + 1) * 512], pt[:, 0:512]
                    )

            # Deferred attn@V / normalize job queue: attn@V for ki is
            # emitted only after scores+exp of ki+LAG, so the in-order PE
            # stream never parks on an exp (or a chunk-boundary psum-ring
            # WAR) while later scores could keep the ACT engine saturated.
            jobs = []
            LAG = 2

            def drain(n_keep):
                while len(jobs) > n_keep:
                    jobs.pop(0)()

            wqk_tiles = {}
            for g in range(N_GROUPS):
                qkt = {}
                wts = {}
                if g % 2 == 0:
                    for name in ("wq", "wk"):
                        wqk_tiles[name] = (
                            wqk_pre[name] if g == 0 else load_wqk2(name, 1)
                        )
                for name in ("wq", "wk"):
                    qkt[name] = qk_pool.tile(
                        [P, SEQ], bf16, tag=f"{name}t", name=f"{name}t"
                    )
                    wts[name] = wqk_views(wqk_tiles[name], g)

                if g == 0:
                    # Software-pipelined startup: each xT s-chunk sj
                    # unlocks V' si 4sj..4sj+3, the QK sj chunk, and
                    # attention qc=0 ki 4sj..4sj+3 (qc0 only needs
                    # qt[:, 0:512] = sj0).  Without this the whole
                    # group-0 projection serializes before the first exp.
                    pass
                else:
                    # ---- V' heads 4-5 (heads 6-7 defer into g2's
                    # q-chunk bubbles; group 3 needs them only later) ----
                    if g == 2:
                        wvt_h1 = load_wv(1)
                    # Only KT sj0/sj1 + QT sj0 up front (all qc0 needs
                    # until ki8); the rest lands on ki-marks below, so the
                    # previous group's bubbles only have to absorb ~5us
                    # of this group's projections instead of ~14us.
                    build_qk_chunk(0, qkt, wts)
                    build_qk_chunk(1, qkt, wts, names=("wk",))
                    if g == 2:
                        build_v2_chunk(range(NS), wvt_h1, 4, 0)

                if g == 1:
                    load_wo()

                # ---- attention for the head pair (2g, 2g+1) ------------
                kt, qt = qkt["wk"], qkt["wq"]

                def score_exp(ki, q0, qw=QC, kt=kt, qt=qt):
                    qs = slice(q0, q0 + qw)
                    sps = ps_s.tile([P, 1024], f32, tag="s", name="sps")
                    ks = slice(ki * P, (ki + 1) * P)
                    # two heads stream concurrently: distinct PE
                    # row-groups (base partitions 0 / 64).  Head B always
                    # lands at column 512 so the two start=True matmuls
                    # never share a psum bank (a start zeroes its whole
                    # 2KB zero-region).
                    nc.tensor.matmul(
                        sps[:, 0:qw],
                        kt[0:HEAD_DIM, ks], qt[0:HEAD_DIM, qs],
                        start=True, stop=True,
                    )
                    nc.tensor.matmul(
                        sps[:, 512:512 + qw],
                        kt[HEAD_DIM:P, ks], qt[HEAD_DIM:P, qs],
                        start=True, stop=True,
                    )
                    et = exp_pool.tile(
                        [P, 1024], bf16, tag="et", name="et", bufs=6
                    )
                    sps_v = sps[:].rearrange(
                        "p (h c) -> p h c", h=2, c=512
                    )[:, :, 0:qw]
                    et_v = et[:, 0:2 * qw].rearrange(
                        "p (h c) -> p h c", h=2, c=qw
                    )
                    nc.scalar.activation(et_v, sps_v, EXP, scale=1.0 / 8.0)
                    return et

                def make_av(et, ki, holder, qw=QC, g=g):
                    def av():
                        apsA, apsB = get_aps(holder)
                        # flipped attn@V: out[q, 65] per q-tile, ap 65.
                        # start=True zeroes the whole 2KB psum zero-
                        # region (bank), so only the FIRST matmul per
                        # aps tile carries it; the other q-tiles' ki==0
                        # matmuls land on pending-zero bytes and
                        # overwrite (not accumulate).
                        for sub, aps in ((0, apsA), (1, apsB)):
                            for qti in range(qw // P):
                                nc.tensor.matmul(
                                    aps[:, qti * P: qti * P + VB],
                                    et[:, sub * qw + qti * P:
                                       sub * qw + (qti + 1) * P],
                                    vp_r[:, ki, 2 * g + sub, 0:VB],
                                    start=(ki == 0 and qti == 0),
                                    stop=(ki == NS - 1),
                                    skip_group_check=True,
                                )
                    return av

                def get_aps(holder):
                    # lazy: the ps_a ring slots must be claimed only when
                    # the first attn@V is emitted (after the previous
                    # chunk's tp transposes), keeping ring order sound
                    if "A" not in holder:
                        holder["A"] = ps_a.tile(
                            [P, QC], f32, tag="a", name="apsA"
                        )
                        holder["B"] = ps_a.tile(
                            [P, QC], f32, tag="a", name="apsB"
                        )
                    return holder["A"], holder["B"]

                def make_norm(holder, q0, nqt=NT4, tail=False, g=g):
                    def norm():
                        apsA, apsB = get_aps(holder)
                        normalize_pair(apsA, apsB, g, q0, nqt, tail=tail)
                    return norm

                # Deferred projection work, emitted one item per ki-mark
                # inside a q-chunk so it lands in the ~400ns exp-wait
                # bubbles instead of stalling a chunk boundary.
                def qk_job(sj, names):
                    return lambda: build_qk_chunk(sj, qkt, wts, names=names)

                deferred = {}
                if g == 0:
                    deferred[1] = [qk_job(2, ("wq",))] + [
                        (lambda s=s: build_v2_chunk([s, s + 1], wvt_pre, 0, 1))
                        for s in range(0, 8, 2)
                    ]
                    deferred[2] = [qk_job(3, ("wq",))] + [
                        (lambda s=s: build_v2_chunk([s, s + 1], wvt_pre, 0, 1))
                        for s in range(8, 16, 2)
                    ]
                else:
                    # kt sj2 lands by ki~6 (needed ki8), kt sj3 by ki~10
                    # (needed ki12), qt sj1 by qc1
                    deferred[0] = [
                        qk_job(2, ("wk",)),
                        qk_job(3, ("wk",)),
                        qk_job(1, ("wq",)),
                    ]
                    deferred[1] = [qk_job(2, ("wq",))]
                    deferred[2] = [qk_job(3, ("wq",))]
                    if g == 2:
                        bv = [
                            (lambda s=s: build_v2_chunk(
                                [s, s + 1], wvt_h1, 4, 1))
                            for s in range(0, 16, 2)
                        ]
                        deferred[1] += bv[0:2]
                        deferred[2] += bv[2:4]
                        deferred[3] = bv[4:8]

                def build_kt_cols(c0, c1):
                    pt = ps_p.tile([P, 512], f32, tag="p", name="pt")
                    for ei in range(NE):
                        nc.tensor.matmul(
                            pt[:, 0:c1 - c0],
                            wts["wk"][ei],
                            xt_blk(ei, c0, c1 - c0),
                            start=(ei == 0),
                            stop=(ei == NE - 1),
                        )
                    nc.vector.tensor_copy(
                        qkt["wk"][:, c0:c1], pt[:, 0:c1 - c0]
                    )

                if g == 0:
                    # software-pipelined startup: qc0 strictly needs KT
                    # (all sj), QT sj0, and V' heads 0-1 -- everything
                    # else (QT sj1-3 for qc1-3, V' heads 2-3 for group 1)
                    # defers into the later q-chunks' exp-wait bubbles.
                    # V' si builds go AFTER score_exp(si): the first exp
                    # fires as soon as KT's first 128 columns land, and
                    # attn@V(ki) only needs V' si=ki at LAG=2.
                    holder = {}
                    for sj in range(4):
                        # each cluster's first scores read only the first
                        # 128 KT columns of its sj chunk: build those
                        # first so the cluster's exp fires early, then the
                        # rest behind the first score_exp
                        build_kt_cols(sj * 512, sj * 512 + P)
                        if sj == 0:
                            build_qk_chunk(0, qkt, wts, names=("wq",))
                        for ki in range(4 * sj, 4 * sj + 4):
                            et = score_exp(ki, 0)
                            jobs.append(make_av(et, ki, holder))
                            if ki % 4 == 0:
                                build_kt_cols(sj * 512 + P, (sj + 1) * 512)
                            build_v2_chunk([ki], wvt_pre, 0, 0)
                            if ki == 9:
                                # QT sj1 (for qc1): xT sj1 has landed by
                                # now; keeps the qc0->qc1 boundary clean
                                build_qk_chunk(1, qkt, wts, names=("wq",))
                            drain(LAG)
                    jobs.append(make_norm(holder, 0))
                    rest_qc = range(1, NQ)
                else:
                    rest_qc = range(NQ)
                # group 3 splits its final q-chunk in two so the first
                # half's normalize+final overlaps the second half's exps
                # and only si14-15 remain after the very last exp
                chunks = [(qci, qci * QC, QC) for qci in rest_qc]
                if False:
                    chunks = chunks[:-1] + [
                        (NQ - 1, (NQ - 1) * QC, QC // 2),
                        (NQ, (NQ - 1) * QC + QC // 2, QC // 2),
                    ]
                for qci, cq0, qw in chunks:
                    holder = {}
                    todo = list(deferred.get(qci, []))
                    for ki in range(NS):
                        et = score_exp(ki, cq0, qw)
                        jobs.append(make_av(et, ki, holder, qw))
                        if todo and ki % 3 == 2:
                            todo.pop(0)()
                        drain(LAG)
                    for fn in todo:
                        fn()
                    jobs.append(make_norm(
                        holder, cq0, qw // P,
                        tail=(g == N_GROUPS - 1 and qci == NQ),
                    ))
                    # bulk partial-final emission: non-gating work the
                    # scheduler floats into the ACT-bound bubbles as
                    # group 3's own outT completes.
                    if (g, qci) == (3, 0):
                        for si in range(8, 12):
                            partial_final(si, 0)
                            partial_final(si, 1)
                    elif (g, qci) == (3, 1):
                        for si in range(12, NS):
                            partial_final(si, 0)
                            partial_final(si, 1)

            drain(0)
            if KDBG:
                nc.sync.dma_start(out=d["dbg_outt"][:], in_=outt_tiles[0][:])
            final_proj(range(0, NS))


def _build_nc(repeats=1):
    import concourse.mybir as mybir
    import concourse.tile as tile
    from concourse import bacc

    f32 = mybir.dt.float32
    bf16 = mybir.dt.bfloat16
    nc = bacc.Bacc(
        "TRN2", target_bir_lowering=False, debug=False, num_devices=N_CORES
    )
    d = {
        "xt": nc.dram_tensor("xt", [EMBED, SEQ], bf16, kind="ExternalInput"),
        "wq": nc.dram_tensor("wq", [EMBED, WCOLS], bf16, kind="ExternalInput"),
        "wk": nc.dram_tensor("wk", [EMBED, WCOLS], bf16, kind="ExternalInput"),
        "wv": nc.dram_tensor("wv", [EMBED, WCOLS], bf16, kind="ExternalInput"),
        "wo": nc.dram_tensor("wo", [WCOLS, EMBED], bf16, kind="ExternalInput"),
        "out": nc.dram_tensor("out", [SEQ, EMBED], bf16, kind="ExternalOutput"),
    }
    import os

    if os.environ.get("KDBG"):
        d["dbg_aps"] = nc.dram_tensor(
            "dbg_aps", [P, 512], f32, kind="ExternalOutput"
        )
        d["dbg_rec"] = nc.dram_tensor(
            "dbg_rec", [P, 4], f32, kind="ExternalOutput"
        )
        d["dbg_stg"] = nc.dram_tensor(
            "dbg_stg", [P, 512], bf16, kind="ExternalOutput"
        )
        d["dbg_outt"] = nc.dram_tensor(
            "dbg_outt", [P, SEQ], bf16, kind="ExternalOutput"
        )
    with tile.TileContext(nc) as tc:
        for _ in range(repeats):
            _emit(nc, tc, tile, mybir, d)
    nc.compile()
    return nc


def _get_nc(repeats=1):
    key = f"nc{repeats}"
    if key not in _cache:
        _cache[key] = _build_nc(repeats)
    return _cache[key]


def make_in_maps(x, Wq, Wk, Wv, Wo, bo):
    import ml_dtypes

    bf16 = ml_dtypes.bfloat16
    x = np.asarray(x, dtype=np.float32)
    Wq = np.asarray(Wq, dtype=np.float32)
    Wk = np.asarray(Wk, dtype=np.float32)
    Wv = np.asarray(Wv, dtype=np.float32)
    Wo = np.asarray(Wo, dtype=np.float32)
    bo = np.asarray(bo, dtype=np.float32)
    xts = [np.ascontiguousarray(x[b].T).astype(bf16) for b in range(BATCH)]
    in_maps = []
    for c in range(N_CORES):
        b, H = c // 2, c % 2
        cs = slice(H * WCOLS, (H + 1) * WCOLS)
        in_maps.append({
            "xt": xts[b],
            "wq": np.ascontiguousarray(Wq[:, cs]).astype(bf16),
            "wk": np.ascontiguousarray(Wk[:, cs]).astype(bf16),
            "wv": np.ascontiguousarray(Wv[:, cs]).astype(bf16),
            "wo": np.ascontiguousarray(Wo[cs, :]).astype(bf16),
        })
    return in_maps


def _get_runner(repeats=1):
    """Cached jitted SPMD callable (avoids per-call retrace)."""
    key = f"runner{repeats}"
    if key in _cache:
        return _cache[key]
    import jax
    from jax.sharding import Mesh, NamedSharding, PartitionSpec
    from jax.experimental.shard_map import shard_map
    from concourse import mybir
    from concourse.bass2jax import (
        _bass_exec_p,
        install_neuronx_cc_hook,
        partition_id_tensor,
    )

    nc = _get_nc(repeats)
    install_neuronx_cc_hook()
    pname = nc.partition_id_tensor.name if nc.partition_id_tensor else None
    in_names, out_names, out_avals, zeros = [], [], [], []
    for alloc in nc.m.functions[0].allocations:
        if not isinstance(alloc, mybir.MemoryLocationSet):
            continue
        name = alloc.memorylocations[0].name
        if alloc.kind == "ExternalInput":
            if name != pname:
                in_names.append(name)
        elif alloc.kind == "ExternalOutput":
            shape = tuple(alloc.tensor_shape)
            dtype = mybir.dt.np(alloc.dtype)
            out_names.append(name)
            out_avals.append(jax.core.ShapedArray(shape, dtype))
            zeros.append(np.zeros(shape, dtype))
    names_all = in_names + out_names + ([pname] if pname else [])

    def _body(*args):
        operands = list(args)
        if pname is not None:
            operands.append(partition_id_tensor())
        return tuple(_bass_exec_p.bind(
            *operands,
            out_avals=tuple(out_avals),
            in_names=tuple(names_all),
            out_names=tuple(out_names),
            lowering_input_output_aliases=(),
            sim_require_finite=True,
            sim_require_nnan=True,
            nc=nc,
        ))

    devices = jax.devices()[:N_CORES]
    mesh = Mesh(np.asarray(devices), ("core",))
    nio = len(in_names) + len(out_names)
    sharded = jax.jit(
        shard_map(
            _body, mesh=mesh,
            in_specs=(PartitionSpec("core"),) * nio,
            out_specs=(PartitionSpec("core"),) * len(out_names),
            check_rep=False,
        ),
        keep_unused=True,
    )
    sh = NamedSharding(mesh, PartitionSpec("core"))
    zdev = [
        jax.device_put(np.zeros((N_CORES * z.shape[0], *z.shape[1:]), z.dtype), sh)
        for z in zeros
    ]
    _cache[key] = (sharded, in_names, out_names, out_avals, zdev, sh)
    return _cache[key]


def kernel(x, Wq, Wk, Wv, Wo, bo, trace=False):
    in_maps = make_in_maps(x, Wq, Wk, Wv, Wo, bo)
    try:
        import jax

        sharded, in_names, out_names, out_avals, zdev, sh = _get_runner()
        concat = [
            jax.device_put(
                np.concatenate([m[n] for m in in_maps], axis=0), sh
            )
            for n in in_names
        ]
        outs = sharded(*concat, *zdev)
        arr = np.asarray(outs[out_names.index("out")]).reshape(
            N_CORES, SEQ, EMBED
        )
        bo32 = np.asarray(bo, dtype=np.float32).reshape(EMBED)
        out = np.empty((BATCH, SEQ, EMBED), dtype=np.float32)
        for b in range(BATCH):
            out[b] = (arr[2 * b].astype(np.float32)
                      + arr[2 * b + 1].astype(np.float32) + bo32)
        return out
    except Exception:
        from concourse.bass_utils import run_bass_kernel_spmd

        nc = _get_nc()
        res = run_bass_kernel_spmd(
            nc, in_maps, list(range(N_CORES)), trace=trace
        )
        _cache["last_result"] = res
        bo32 = np.asarray(bo, dtype=np.float32).reshape(EMBED)
        out = np.empty((BATCH, SEQ, EMBED), dtype=np.float32)
        for b in range(BATCH):
            out[b] = (res.results[2 * b]["out"].astype(np.float32)
                      + res.results[2 * b + 1]["out"].astype(np.float32)
                      + bo32)
        return out


# revision 74
# speedup vs baseline: 1.0017x; 1.0017x over previous
"""Multi-head attention (dense transformer block) on 8 TRN2 NeuronCores.

Sharding: 8 cores = 4 batches x 2 head-halves.
  core c: batch b = c // 2, head half H = c % 2 (heads H*8 .. H*8+8).
  Each core computes attention for its 8 heads of its batch plus the
  partial final projection (row-shard of Wo); the host sums core pairs
  and adds the output bias in the same epilogue.

Per-core kernel. All SBUF-resident operands are bf16 (halves the input
DMA and enables FWL weight loads); PSUM accumulation stays fp32.

  0. Load pre-transposed x^T (host supplies bf16 x^T) into SBUF [e, s].
  1. Per 2-head group g: QT_g/KT_g [128, 2048] in [d, s] layout
     (wq/wk projections emitted sj-interleaved so attention's first ki
     chunks unblock early); V for 4 heads at a time in [s, d+1] layout
     with a ones column per head block.
  2. Attention per head PAIR (2g, 2g+1) and q-chunk of 512:
     scoresT [k, q]: two K=64 matmuls (base partitions 0/64) into one
     pair psum tile [128, (headA 512 | headB 512)], one ACT exp ->
     bf16 et (scale=1/8; no max subtraction: |score/8| <~ 6).
     attn@V runs FLIPPED: out[q, d] with et 128-q slices as lhsT and
     V' [k, 65] as rhs -> 8 matmuls of ap_size 65 per ki instead of 2
     of 512 (PE cost is output-free-size per accumulation step, so
     this halves attn@V PE time).  The ones column of V' lands the
     softmax denominator at column 64 of the SAME partition as its q
     row.  Only the first matmul per aps tile carries start=True (a
     start zeroes the whole 2KB psum zero-region); the other q-tiles'
     ki==0 matmuls overwrite via the pending-zero bytes.
     attn@V trails scores/exp by LAG=2 ki steps (a deferred job
     queue), so the in-order PE stream never parks on an exp wait
     while later scores could keep ACT (the 1038ns/ki bottleneck
     engine) saturated -- this also pipelines chunk and group
     boundaries through the normalize's psum-ring WAR.
  3. Normalize off the critical path: one DVE reciprocal per head
     (4 denominators via a strided psum view), then per q-tile a
     per-partition tensor_scalar multiply into a [128, (dA|dB)] bf16
     staging tile; ONE identity matmul transposes both heads' tiles
     back to [d, q] (psum), evacuated into outT[g] [hd, s] by DVE.
     The very last chunk routes alternate ops through the then-idle
     ACT engine.
  4. final: out[s,e] = sum_hd outT[hd,s]^T @ Wo[hd,e] -> DRAM in bf16
     (bias add + f32 upcast in the host pair-sum epilogue).  Groups
     0-2's contribution to the last four s-tiles is pre-accumulated
     into SBUF partials during group 3's exp-wait bubbles and
     re-injected via an identity matmul, so the post-attention tail
     only runs group 3's matmul per tile.

  Scheduling: the tile scheduler hoists ready work into PE bubbles,
  so each group builds only KT sj0/sj1 + QT sj0 up front and defers
  KT sj2/3 + QT sj1-3 (and V' half-builds) onto explicit ki-marks
  inside its own chunks; group 0 software-pipelines its first chunk
  against the 4 big per-sj xT DMAs (issued critical-first).
"""

import numpy as np

EMBED = 1024
HEADS = 16
HEAD_DIM = 64
SEQ = 2048
BATCH = 4
N_CORES = 8

LOCAL_HEADS = 8
N_GROUPS = 4
WCOLS = LOCAL_HEADS * HEAD_DIM  # 512

P = 128
NS = SEQ // P    # 16
NE = EMBED // P  # 8
VB = HEAD_DIM + 1  # 65
QC = 512         # q-chunk
NQ = SEQ // QC   # 4
NT4 = QC // P    # 4 q-tiles per q-chunk

TIMING_REPEATS = 16

_cache = {}


def _emit(nc, tc, tile, mybir, d):
    import os

    from concourse import masks

    f32 = mybir.dt.float32
    bf16 = mybir.dt.bfloat16
    EXP = mybir.ActivationFunctionType.Exp
    KDBG = bool(os.environ.get("KDBG")) and "dbg_aps" in d

    with (
        tc.tile_pool(name="const", bufs=1) as const_pool,
        tc.tile_pool(name="xt", bufs=1) as xt_pool,
        tc.tile_pool(name="v", bufs=1) as v_pool,
        tc.tile_pool(name="qk", bufs=2) as qk_pool,
        tc.tile_pool(name="wst", bufs=1) as wst_pool,
        tc.tile_pool(name="ps_s", bufs=2, space="PSUM") as ps_s,
        tc.tile_pool(name="ps_p", bufs=2, space="PSUM") as ps_p,
        tc.tile_pool(name="ps_a", bufs=2, space="PSUM") as ps_a,
    ):
        def load_wv(half):
            wvt = wst_pool.tile([P, NE * 256], bf16, tag="wv", name="wvt")
            wv_v = d["wv"][:].rearrange("(e p) c -> p e c", e=NE, p=P)
            nc.sync.dma_start(
                out=wvt[:].rearrange("p (e c) -> p e c", e=NE, c=256),
                in_=wv_v[:, :, half * 256:(half + 1) * 256],
            )
            return wvt

        def load_wqk2(name, pair):
            """One DMA loads wq/wk columns for TWO groups (256 cols): the
            512B contiguous runs dodge the <512B DMA read-modify-write
            penalty and halve the serialized HWDGE issue count."""
            wt = wst_pool.tile(
                [P, NE * 256], bf16, tag="wqk", bufs=2, name="wqk"
            )
            w_v = d[name][:].rearrange("(e p) c -> p e c", e=NE, p=P)
            nc.sync.dma_start(
                out=wt[:].rearrange("p (e c) -> p e c", e=NE, c=256),
                in_=w_v[:, :, pair * 256:(pair + 1) * 256],
            )
            return wt

        def wqk_views(wt, g):
            sub = g % 2
            return [
                wt[:, ei * 256 + sub * P: ei * 256 + (sub + 1) * P]
                for ei in range(NE)
            ]

        xt_big = xt_pool.tile([P, NE * SEQ], bf16, tag="xt", name="xt_big")
        xt_view = xt_big[:].rearrange("p (e s) -> p e s", e=NE, s=SEQ)
        x_dram = d["xt"][:].rearrange("(e p) s -> p e s", e=NE, p=P)

        def load_xt(sj):
            # one 1MB DMA per s-chunk: all 8 e-blocks at once (1KB runs)
            nc.sync.dma_start(
                out=xt_view[:, :, sj * 512:(sj + 1) * 512],
                in_=x_dram[:, :, sj * 512:(sj + 1) * 512],
            )

        # Critical-path-first DMA order (HWDGE issue AND the transfer
        # engine pool are serialized): wk then xT chunk 0 gate the first
        # KT build -> first scores; wq next (QT sj0), then V weights,
        # then the remaining xT chunks.
        load_xt(0)
        wqk_pre = {"wk": load_wqk2("wk", 0)}
        wqk_pre["wq"] = load_wqk2("wq", 0)
        wvt_pre = load_wv(0)
        for sj in range(1, 4):
            load_xt(sj)

        def xt_blk(ei, s0, slen):
            return xt_big[:, ei * SEQ + s0: ei * SEQ + s0 + slen]

        # V': [p][si][h][b=65] bf16, ones at col 64
        vp = v_pool.tile([P, NS * LOCAL_HEADS * VB], bf16, tag="vp", name="vp")
        vp_r = vp[:].rearrange(
            "p (s h b) -> p s h b", s=NS, h=LOCAL_HEADS, b=VB
        )
        ones_blocks = NS * LOCAL_HEADS
        ones_view = vp[:].rearrange(
            "p (blk c) -> p blk c", blk=ones_blocks, c=VB
        )[:, :, HEAD_DIM:HEAD_DIM + 1]

        ones128 = const_pool.tile([P, P], f32, tag="ones", name="ones128")
        nc.gpsimd.memset(ones128[:], 1.0)
        ident = const_pool.tile([P, P], bf16, tag="ident", name="ident")
        masks.make_identity(nc, ident[:])
        # warm the ACT exp table set (~2.7us ACT_TABLE_LOAD) during the
        # DMA-bound startup instead of at the first real softmax exp.
        warm = const_pool.tile([1, 1], f32, tag="warm", name="warm")
        nc.scalar.activation(warm[:], ones128[0:1, 0:1], EXP)
        nc.vector.tensor_copy(
            ones_view,
            ones128[:].rearrange("p (a b) -> p a b", a=P, b=1)[
                :, 0:ones_blocks, :
            ],
        )

        # PE p-state warmup: burn the 3us cold-clock ramp on dummy identity
        # matmuls while the first DMAs are in flight.
        wps = ps_s.tile([P, 1024], f32, tag="s", name="wps")
        for _ in range(52):
            nc.tensor.matmul(
                wps[:, 0:P], ident[:], ident[:],
                start=True, stop=True, skip_group_check=True,
            )

        with (
            tc.tile_pool(name="exp", bufs=1) as exp_pool,
            tc.tile_pool(name="small", bufs=1) as small_pool,
            tc.tile_pool(name="outt", bufs=1) as outt_pool,
        ):
            outt_tiles = [
                outt_pool.tile([P, SEQ], bf16, tag=f"outt{g}", name=f"outt{g}")
                for g in range(N_GROUPS)
            ]

            # dedicated wo tiles, loaded at group 2 so the partial final
            # projections can start as soon as groups 0-2's outT is done
            wo_tiles = []

            def load_wo():
                for j in range(2):
                    wo = wst_pool.tile(
                        [P, SEQ], bf16, tag="wo", name="wo", bufs=2
                    )
                    wo_tiles.append(wo)
                    for jj in range(2):
                        c = 2 * j + jj
                        nc.sync.dma_start(
                            out=wo[:, jj * 1024:(jj + 1) * 1024],
                            in_=d["wo"][c * P:(c + 1) * P, :],
                        )

            # Tail shortener: groups 0-2's contribution to the LAST four
            # s-tiles is pre-accumulated into SBUF during group 3's
            # exp-wait bubbles; the post-attention tail then only runs the
            # group-3 matmul + a DVE add per (si, ej).
            partials = {}

            def partial_final(si, ej):
                pt = ps_p.tile([P, 512], f32, tag="p", name="pt")
                for c in range(3):
                    nc.tensor.matmul(
                        pt[:, 0:512],
                        outt_tiles[c][:, si * P:(si + 1) * P],
                        wo_tiles[c // 2][:, (c % 2) * 1024 + ej * 512:
                                         (c % 2) * 1024 + (ej + 1) * 512],
                        start=(c == 0),
                        stop=(c == 2),
                        skip_group_check=True,
                    )
                part = small_pool.tile(
                    [P, 512], bf16, tag="part", name="part", bufs=16
                )
                nc.vector.tensor_copy(part[:], pt[:, 0:512])
                partials[(si, ej)] = part

            def final_proj(si_range):
                # ps_p only: the ps_s ring is serialized behind the whole
                # attention stream, which would block the si<8 half from
                # weaving into group 3's ACT-bound bubbles.  Stores go out
                # per ej half so the last DMA chain starts earlier.
                for si in si_range:
                    ot = exp_pool.tile(
                        [P, 1024], bf16, tag="ot", name="ot", bufs=8
                    )
                    for ej in range(2):
                        part = partials.get((si, ej))
                        if part is not None and ej:
                            # scores psum is free at the tail: widen the
                            # effective pt ring so evacs don't serialize
                            # behind the 2-slot ps_p ring
                            pt = ps_s.tile(
                                [P, 1024], f32, tag="s", name="pts"
                            )[:, 0:512]
                        else:
                            pt = ps_p.tile([P, 512], f32, tag="p", name="pt")
                        if part is not None:
                            # inject the groups-0..2 partial into the
                            # accumulation group: identity @ part == part
                            nc.tensor.matmul(
                                pt[:, 0:512],
                                ident[:],
                                part[:],
                                start=True,
                                stop=False,
                                skip_group_check=True,
                            )
                        crange = range(3, 4) if part is not None else range(4)
                        # bias is folded into the host pair-sum epilogue
                        for c in crange:
                            nc.tensor.matmul(
                                pt[:, 0:512],
                                outt_tiles[c][:, si * P:(si + 1) * P],
                                wo_tiles[c // 2][:, (c % 2) * 1024 + ej * 512:
                                                 (c % 2) * 1024 + (ej + 1) * 512],
                                start=(part is None and c == 0),
                                stop=(c == 3),
                                skip_group_check=True,
                            )
                        if part is not None:
                            # ACT is idle after the last exp: evacuate the
                            # tail halves there so DVE isn't the tail chain
                            eng = nc.vector.tensor_copy if ej == 0 else (
                                lambda o, i: nc.scalar.activation(
                                    o, i, mybir.ActivationFunctionType.Copy
                                )
                            )
                            eng(ot[:, ej * 512:(ej + 1) * 512], pt[:, 0:512])
                        else:
                            nc.vector.tensor_copy(
                                ot[:, ej * 512:(ej + 1) * 512], pt[:, 0:512]
                            )
                        nc.sync.dma_start(
                            out=d["out"][si * P:(si + 1) * P,
                                         ej * 512:(ej + 1) * 512],
                            in_=ot[:, ej * 512:(ej + 1) * 512],
                        )

            def normalize_pair(apsA, apsB, g, q0, nqt=NT4, tail=False):
                """Normalize + transpose both heads of the pair for one
                q-chunk.  aps tiles are [128, 512] psum: per q-tile qt a
                [128, 65] region at column qt*128 (col 64 = denominator,
                same partition as its q row)."""
                dbg_here = KDBG and g == 0 and q0 == QC
                recs = []
                for aps in (apsA, apsB):
                    rec = small_pool.tile(
                        [P, NT4], f32, tag="rec", name="rec", bufs=6
                    )
                    dview = aps[:].rearrange(
                        "p (q c) -> p q c", q=NT4, c=P
                    )[:, 0:nqt, HEAD_DIM:HEAD_DIM + 1].rearrange(
                        "p q c -> p (q c)"
                    )
                    nc.vector.reciprocal(rec[:, 0:nqt], dview)
                    recs.append(rec)
                if dbg_here:
                    nc.sync.dma_start(out=d["dbg_rec"][:], in_=recs[0][:])
                stgs = []
                for qt in range(nqt):
                    stg = small_pool.tile(
                        [P, P], bf16, tag="stg", name="stg", bufs=10
                    )
                    # tail (very last chunk): odd q-tiles ride the now-
                    # idle ACT engine (Copy with per-partition scale) so
                    # the post-attention normalize chain isn't DVE-serial
                    for sub, (aps, rec) in enumerate(zip((apsA, apsB), recs)):
                        if tail and qt % 2:
                            nc.scalar.activation(
                                stg[:, sub * HEAD_DIM:(sub + 1) * HEAD_DIM],
                                aps[:, qt * P:qt * P + HEAD_DIM],
                                mybir.ActivationFunctionType.Copy,
                                scale=rec[:, qt:qt + 1],
                            )
                        else:
                            nc.vector.tensor_scalar_mul(
                                stg[:, sub * HEAD_DIM:(sub + 1) * HEAD_DIM],
                                aps[:, qt * P:qt * P + HEAD_DIM],
                                rec[:, qt:qt + 1],
                            )
                    stgs.append(stg)
                if dbg_here:
                    for qt in range(NT4):
                        nc.sync.dma_start(
                            out=d["dbg_stg"][:, qt * P:(qt + 1) * P],
                            in_=stgs[qt][:],
                        )
                for qt in range(nqt):
                    # one identity matmul transposes both heads' 64-col
                    # halves: out rows 0..63 = headA^T, 64..127 = headB^T
                    tp = ps_a.tile([P, 512], f32, tag="a", name="tp")
                    nc.tensor.matmul(
                        tp[:, 0:P],
                        stgs[qt][:],
                        ident[:],
                        start=True,
                        stop=True,
                        skip_group_check=True,
                    )
                    if tail and qt % 2:
                        nc.scalar.activation(
                            outt_tiles[g][:, q0 + qt * P: q0 + (qt + 1) * P],
                            tp[:, 0:P],
                            mybir.ActivationFunctionType.Copy,
                        )
                    else:
                        nc.vector.tensor_copy(
                            outt_tiles[g][:, q0 + qt * P: q0 + (qt + 1) * P],
                            tp[:, 0:P],
                        )

            def build_v_chunk(si_range, wvt, h0):
                for si in si_range:
                    pt = ps_p.tile([P, 512], f32, tag="p", name="pt")
                    for ei in range(NE):
                        nc.tensor.matmul(
                            pt[:, 0:256],
                            xt_blk(ei, si * P, P),
                            wvt[:, ei * 256:(ei + 1) * 256],
                            start=(ei == 0),
                            stop=(ei == NE - 1),
                        )
                    dst = vp_r[:, si, h0:h0 + 4, 0:HEAD_DIM]
                    nc.vector.tensor_copy(
                        dst,
                        pt[:, 0:256].rearrange(
                            "p (h b) -> p h b", h=4, b=HEAD_DIM
                        ),
                    )

            def build_v2_chunk(si_range, wvt, h0, hh):
                # two-head V' build: halves the PE cost on group 0's
                # PE-bound startup path (heads 2-3 defer to qc1/qc2)
                for si in si_range:
                    pt = ps_p.tile([P, 512], f32, tag="p", name="pt")
                    for ei in range(NE):
                        nc.tensor.matmul(
                            pt[:, 0:P],
                            xt_blk(ei, si * P, P),
                            wvt[:, ei * 256 + hh * P:
                                ei * 256 + (hh + 1) * P],
                            start=(ei == 0),
                            stop=(ei == NE - 1),
                        )
                    dst = vp_r[:, si, h0 + 2 * hh:h0 + 2 * hh + 2, 0:HEAD_DIM]
                    nc.vector.tensor_copy(
                        dst,
                        pt[:, 0:P].rearrange(
                            "p (h b) -> p h b", h=2, b=HEAD_DIM
                        ),
                    )

            def build_qk_chunk(sj, qkt, wts, names=("wk", "wq")):
                for name in names:
                    pt = ps_p.tile([P, 512], f32, tag="p", name="pt")
                    for ei in range(NE):
                        nc.tensor.matmul(
                            pt[:, 0:512],
                            wts[name][ei],
                            xt_blk(ei, sj * 512, 512),
                            start=(ei == 0),
                            stop=(ei == NE - 1),
                        )
                    nc.vector.tensor_copy(
                        qkt[name][:, sj * 512:(sj + 1) * 512], pt[:, 0:512]
                    )

            # Deferred attn@V / normalize job queue: attn@V for ki is
            # emitted only after scores+exp of ki+LAG, so the in-order PE
            # stream never parks on an exp (or a chunk-boundary psum-ring
            # WAR) while later scores could keep the ACT engine saturated.
            jobs = []
            LAG = 2

            def drain(n_keep):
                while len(jobs) > n_keep:
                    jobs.pop(0)()

            wqk_tiles = {}
            for g in range(N_GROUPS):
                qkt = {}
                wts = {}
                if g % 2 == 0:
                    for name in ("wq", "wk"):
                        wqk_tiles[name] = (
                            wqk_pre[name] if g == 0 else load_wqk2(name, 1)
                        )
                for name in ("wq", "wk"):
                    qkt[name] = qk_pool.tile(
                        [P, SEQ], bf16, tag=f"{name}t", name=f"{name}t"
                    )
                    wts[name] = wqk_views(wqk_tiles[name], g)

                if g == 0:
                    # Software-pipelined startup: each xT s-chunk sj
                    # unlocks V' si 4sj..4sj+3, the QK sj chunk, and
                    # attention qc=0 ki 4sj..4sj+3 (qc0 only needs
                    # qt[:, 0:512] = sj0).  Without this the whole
                    # group-0 projection serializes before the first exp.
                    pass
                else:
                    # ---- V' heads 4-5 (heads 6-7 defer into g2's
                    # q-chunk bubbles; group 3 needs them only later) ----
                    if g == 2:
                        wvt_h1 = load_wv(1)
                    # Only KT sj0/sj1 + QT sj0 up front (all qc0 needs
                    # until ki8); the rest lands on ki-marks below, so the
                    # previous group's bubbles only have to absorb ~5us
                    # of this group's projections instead of ~14us.
                    build_qk_chunk(0, qkt, wts)
                    build_qk_chunk(1, qkt, wts, names=("wk",))
                    if g == 2:
                        build_v2_chunk(range(NS), wvt_h1, 4, 0)

                if g == 1:
                    load_wo()

                # ---- attention for the head pair (2g, 2g+1) ------------
                kt, qt = qkt["wk"], qkt["wq"]

                def score_exp(ki, q0, qw=QC, kt=kt, qt=qt):
                    qs = slice(q0, q0 + qw)
                    sps = ps_s.tile([P, 1024], f32, tag="s", name="sps")
                    ks = slice(ki * P, (ki + 1) * P)
                    # two heads stream concurrently: distinct PE
                    # row-groups (base partitions 0 / 64).  Head B always
                    # lands at column 512 so the two start=True matmuls
                    # never share a psum bank (a start zeroes its whole
                    # 2KB zero-region).
                    nc.tensor.matmul(
                        sps[:, 0:qw],
                        kt[0:HEAD_DIM, ks], qt[0:HEAD_DIM, qs],
                        start=True, stop=True,
                    )
                    nc.tensor.matmul(
                        sps[:, 512:512 + qw],
                        kt[HEAD_DIM:P, ks], qt[HEAD_DIM:P, qs],
                        start=True, stop=True,
                    )
                    et = exp_pool.tile(
                        [P, 1024], bf16, tag="et", name="et", bufs=6
                    )
                    sps_v = sps[:].rearrange(
                        "p (h c) -> p h c", h=2, c=512
                    )[:, :, 0:qw]
                    et_v = et[:, 0:2 * qw].rearrange(
                        "p (h c) -> p h c", h=2, c=qw
                    )
                    nc.scalar.activation(et_v, sps_v, EXP, scale=1.0 / 8.0)
                    return et

                def make_av(et, ki, holder, qw=QC, g=g):
                    def av():
                        apsA, apsB = get_aps(holder)
                        # flipped attn@V: out[q, 65] per q-tile, ap 65.
                        # start=True zeroes the whole 2KB psum zero-
                        # region (bank), so only the FIRST matmul per
                        # aps tile carries it; the other q-tiles' ki==0
                        # matmuls land on pending-zero bytes and
                        # overwrite (not accumulate).
                        for sub, aps in ((0, apsA), (1, apsB)):
                            for qti in range(qw // P):
                                nc.tensor.matmul(
                                    aps[:, qti * P: qti * P + VB],
                                    et[:, sub * qw + qti * P:
                                       sub * qw + (qti + 1) * P],
                                    vp_r[:, ki, 2 * g + sub, 0:VB],
                                    start=(ki == 0 and qti == 0),
                                    stop=(ki == NS - 1),
                                    skip_group_check=True,
                                )
                    return av

                def get_aps(holder):
                    # lazy: the ps_a ring slots must be claimed only when
                    # the first attn@V is emitted (after the previous
                    # chunk's tp transposes), keeping ring order sound
                    if "A" not in holder:
                        holder["A"] = ps_a.tile(
                            [P, QC], f32, tag="a", name="apsA"
                        )
                        holder["B"] = ps_a.tile(
                            [P, QC], f32, tag="a", name="apsB"
                        )
                    return holder["A"], holder["B"]

                def make_norm(holder, q0, nqt=NT4, tail=False, g=g):
                    def norm():
                        apsA, apsB = get_aps(holder)
                        normalize_pair(apsA, apsB, g, q0, nqt, tail=tail)
                    return norm

                # Deferred projection work, emitted one item per ki-mark
                # inside a q-chunk so it lands in the ~400ns exp-wait
                # bubbles instead of stalling a chunk boundary.
                def qk_job(sj, names):
                    return lambda: build_qk_chunk(sj, qkt, wts, names=names)

                deferred = {}
                if g == 0:
                    deferred[1] = [qk_job(2, ("wq",))] + [
                        (lambda s=s: build_v2_chunk([s, s + 1], wvt_pre, 0, 1))
                        for s in range(0, 8, 2)
                    ]
                    deferred[2] = [qk_job(3, ("wq",))] + [
                        (lambda s=s: build_v2_chunk([s, s + 1], wvt_pre, 0, 1))
                        for s in range(8, 16, 2)
                    ]
                else:
                    # kt sj2 lands by ki~6 (needed ki8), kt sj3 by ki~10
                    # (needed ki12), qt sj1 by qc1
                    deferred[0] = [
                        qk_job(2, ("wk",)),
                        qk_job(3, ("wk",)),
                        qk_job(1, ("wq",)),
                    ]
                    deferred[1] = [qk_job(2, ("wq",))]
                    deferred[2] = [qk_job(3, ("wq",))]
                    if g == 2:
                        bv = [
                            (lambda s=s: build_v2_chunk(
                                [s, s + 1], wvt_h1, 4, 1))
                            for s in range(0, 16, 2)
                        ]
                        deferred[1] += bv[0:2]
                        deferred[2] += bv[2:4]
                        deferred[3] = bv[4:8]

                def build_kt_cols(c0, c1):
                    pt = ps_p.tile([P, 512], f32, tag="p", name="pt")
                    for ei in range(NE):
                        nc.tensor.matmul(
                            pt[:, 0:c1 - c0],
                            wts["wk"][ei],
                            xt_blk(ei, c0, c1 - c0),
                            start=(ei == 0),
                            stop=(ei == NE - 1),
                        )
                    nc.vector.tensor_copy(
                        qkt["wk"][:, c0:c1], pt[:, 0:c1 - c0]
                    )

                if g == 0:
                    # software-pipelined startup: qc0 strictly needs KT
                    # (all sj), QT sj0, and V' heads 0-1 -- everything
                    # else (QT sj1-3 for qc1-3, V' heads 2-3 for group 1)
                    # defers into the later q-chunks' exp-wait bubbles.
                    # V' si builds go AFTER score_exp(si): the first exp
                    # fires as soon as KT's first 128 columns land, and
                    # attn@V(ki) only needs V' si=ki at LAG=2.
                    holder = {}
                    for sj in range(4):
                        # each cluster's first scores read only the first
                        # 128 KT columns of its sj chunk: build those
                        # first so the cluster's exp fires early, then the
                        # rest behind the first score_exp
                        build_kt_cols(sj * 512, sj * 512 + P)
                        if sj == 0:
                            build_qk_chunk(0, qkt, wts, names=("wq",))
                        for ki in range(4 * sj, 4 * sj + 4):
                            et = score_exp(ki, 0)
                            jobs.append(make_av(et, ki, holder))
                            if ki % 4 == 0:
                                build_kt_cols(sj * 512 + P, (sj + 1) * 512)
                            build_v2_chunk([ki], wvt_pre, 0, 0)
                            if ki == 9:
                                # QT sj1 (for qc1): xT sj1 has landed by
                                # now; keeps the qc0->qc1 boundary clean
                                build_qk_chunk(1, qkt, wts, names=("wq",))
                            drain(LAG)
                    jobs.append(make_norm(holder, 0))
                    rest_qc = range(1, NQ)
                else:
                    rest_qc = range(NQ)
                # group 3 splits its final q-chunk in two so the first
                # half's normalize+final overlaps the second half's exps
                # and only si14-15 remain after the very last exp
                chunks = [(qci, qci * QC, QC) for qci in rest_qc]
                if False:
                    chunks = chunks[:-1] + [
                        (NQ - 1, (NQ - 1) * QC, QC // 2),
                        (NQ, (NQ - 1) * QC + QC // 2, QC // 2),
                    ]
                for qci, cq0, qw in chunks:
                    holder = {}
                    todo = list(deferred.get(qci, []))
                    for ki in range(NS):
                        et = score_exp(ki, cq0, qw)
                        jobs.append(make_av(et, ki, holder, qw))
                        if todo and ki % 3 == 2:
                            todo.pop(0)()
                        # group 3's normalize/final region is DVE-crowded:
                        # deeper lag buys its chunk boundaries more runway
                        drain(LAG if g < 3 else 4)
                    for fn in todo:
                        fn()
                    jobs.append(make_norm(
                        holder, cq0, qw // P,
                        tail=(g == N_GROUPS - 1 and qci == NQ),
                    ))
                    # bulk partial-final emission: non-gating work the
                    # scheduler floats into the ACT-bound bubbles as
                    # group 3's own outT completes.
                    if (g, qci) == (3, 0):
                        for si in range(8, 12):
                            partial_final(si, 0)
                            partial_final(si, 1)
                    elif (g, qci) == (3, 1):
                        for si in range(12, NS):
                            partial_final(si, 0)
                            partial_final(si, 1)

            drain(0)
            if KDBG:
                nc.sync.dma_start(out=d["dbg_outt"][:], in_=outt_tiles[0][:])
            final_proj(range(0, NS))


def _build_nc(repeats=1):
    import concourse.mybir as mybir
    import concourse.tile as tile
    from concourse import bacc

    f32 = mybir.dt.float32
    bf16 = mybir.dt.bfloat16
    nc = bacc.Bacc(
        "TRN2", target_bir_lowering=False, debug=False, num_devices=N_CORES
    )
    d = {
        "xt": nc.dram_tensor("xt", [EMBED, SEQ], bf16, kind="ExternalInput"),
        "wq": nc.dram_tensor("wq", [EMBED, WCOLS], bf16, kind="ExternalInput"),
        "wk": nc.dram_tensor("wk", [EMBED, WCOLS], bf16, kind="ExternalInput"),
        "wv": nc.dram_tensor("wv", [EMBED, WCOLS], bf16, kind="ExternalInput"),
        "wo": nc.dram_tensor("wo", [WCOLS, EMBED], bf16, kind="ExternalInput"),
        "out": nc.dram_tensor("out", [SEQ, EMBED], bf16, kind="ExternalOutput"),
    }
    import os

    if os.environ.get("KDBG"):
        d["dbg_aps"] = nc.dram_tensor(
            "dbg_aps", [P, 512], f32, kind="ExternalOutput"
        )
        d["dbg_rec"] = nc.dram_tensor(
            "dbg_rec", [P, 4], f32, kind="ExternalOutput"
        )
        d["dbg_stg"] = nc.dram_tensor(
            "dbg_stg", [P, 512], bf16, kind="ExternalOutput"
        )
        d["dbg_outt"] = nc.dram_tensor(
            "dbg_outt", [P, SEQ], bf16, kind="ExternalOutput"
        )
    with tile.TileContext(nc) as tc:
        for _ in range(repeats):
            _emit(nc, tc, tile, mybir, d)
    nc.compile()
    return nc


def _get_nc(repeats=1):
    key = f"nc{repeats}"
    if key not in _cache:
        _cache[key] = _build_nc(repeats)
    return _cache[key]


def make_in_maps(x, Wq, Wk, Wv, Wo, bo):
    import ml_dtypes

    bf16 = ml_dtypes.bfloat16
    x = np.asarray(x, dtype=np.float32)
    Wq = np.asarray(Wq, dtype=np.float32)
    Wk = np.asarray(Wk, dtype=np.float32)
    Wv = np.asarray(Wv, dtype=np.float32)
    Wo = np.asarray(Wo, dtype=np.float32)
    bo = np.asarray(bo, dtype=np.float32)
    xts = [np.ascontiguousarray(x[b].T).astype(bf16) for b in range(BATCH)]
    in_maps = []
    for c in range(N_CORES):
        b, H = c // 2, c % 2
        cs = slice(H * WCOLS, (H + 1) * WCOLS)
        in_maps.append({
            "xt": xts[b],
            "wq": np.ascontiguousarray(Wq[:, cs]).astype(bf16),
            "wk": np.ascontiguousarray(Wk[:, cs]).astype(bf16),
            "wv": np.ascontiguousarray(Wv[:, cs]).astype(bf16),
            "wo": np.ascontiguousarray(Wo[cs, :]).astype(bf16),
        })
    return in_maps


def _get_runner(repeats=1):
    """Cached jitted SPMD callable (avoids per-call retrace)."""
    key = f"runner{repeats}"
    if key in _cache:
        return _cache[key]
    import jax
    from jax.sharding import Mesh, NamedSharding, PartitionSpec
    from jax.experimental.shard_map import shard_map
    from concourse import mybir
    from concourse.bass2jax import (
        _bass_exec_p,
        install_neuronx_cc_hook,
        partition_id_tensor,
    )

    nc = _get_nc(repeats)
    install_neuronx_cc_hook()
    pname = nc.partition_id_tensor.name if nc.partition_id_tensor else None
    in_names, out_names, out_avals, zeros = [], [], [], []
    for alloc in nc.m.functions[0].allocations:
        if not isinstance(alloc, mybir.MemoryLocationSet):
            continue
        name = alloc.memorylocations[0].name
        if alloc.kind == "ExternalInput":
            if name != pname:
                in_names.append(name)
        elif alloc.kind == "ExternalOutput":
            shape = tuple(alloc.tensor_shape)
            dtype = mybir.dt.np(alloc.dtype)
            out_names.append(name)
            out_avals.append(jax.core.ShapedArray(shape, dtype))
            zeros.append(np.zeros(shape, dtype))
    names_all = in_names + out_names + ([pname] if pname else [])

    def _body(*args):
        operands = list(args)
        if pname is not None:
            operands.append(partition_id_tensor())
        return tuple(_bass_exec_p.bind(
            *operands,
            out_avals=tuple(out_avals),
            in_names=tuple(names_all),
            out_names=tuple(out_names),
            lowering_input_output_aliases=(),
            sim_require_finite=True,
            sim_require_nnan=True,
            nc=nc,
        ))

    devices = jax.devices()[:N_CORES]
    mesh = Mesh(np.asarray(devices), ("core",))
    nio = len(in_names) + len(out_names)
    sharded = jax.jit(
        shard_map(
            _body, mesh=mesh,
            in_specs=(PartitionSpec("core"),) * nio,
            out_specs=(PartitionSpec("core"),) * len(out_names),
            check_rep=False,
        ),
        keep_unused=True,
    )
    sh = NamedSharding(mesh, PartitionSpec("core"))
    zdev = [
        jax.device_put(np.zeros((N_CORES * z.shape[0], *z.shape[1:]), z.dtype), sh)
        for z in zeros
    ]
    _cache[key] = (sharded, in_names, out_names, out_avals, zdev, sh)
    return _cache[key]


def kernel(x, Wq, Wk, Wv, Wo, bo, trace=False):
    in_maps = make_in_maps(x, Wq, Wk, Wv, Wo, bo)
    try:
        import jax

        sharded, in_names, out_names, out_avals, zdev, sh = _get_runner()
        concat = [
            jax.device_put(
                np.concatenate([m[n] for m in in_maps], axis=0), sh
            )
            for n in in_names
        ]
        outs = sharded(*concat, *zdev)
        arr = np.asarray(outs[out_names.index("out")]).reshape(
            N_CORES, SEQ, EMBED
        )
        bo32 = np.asarray(bo, dtype=np.float32).reshape(EMBED)
        out = np.empty((BATCH, SEQ, EMBED), dtype=np.float32)
        for b in range(BATCH):
            out[b] = (arr[2 * b].astype(np.float32)
                      + arr[2 * b + 1].astype(np.float32) + bo32)
        return out
    except Exception:
        from concourse.bass_utils import run_bass_kernel_spmd

        nc = _get_nc()
        res = run_bass_kernel_spmd(
            nc, in_maps, list(range(N_CORES)), trace=trace
        )
        _cache["last_result"] = res
        bo32 = np.asarray(bo, dtype=np.float32).reshape(EMBED)
        out = np.empty((BATCH, SEQ, EMBED), dtype=np.float32)
        for b in range(BATCH):
            out[b] = (res.results[2 * b]["out"].astype(np.float32)
                      + res.results[2 * b + 1]["out"].astype(np.float32)
                      + bo32)
        return out
